# revision 5
# baseline (speedup 1.0000x reference)
"""GroupingPool2d kernel for Trainium2 (8 NeuronCores, Bass/Tile).

The reference module (2x2 non-overlapping windows, min-max normalize,
product-group, denormalize) reduces bitwise-exactly to a 2x2 min-pool:
the window minimum normalizes to exactly 0.0, so the product over the
window is exactly 0.0 and out = 0*(mx-mn)+mn = mn.

Strategy: pure data parallel. Shard batch 16 -> 2 per core; per core
flatten (B=2, C=64) -> 128 SBUF partitions, each partition holding one
384x384 image. Stream row-tiles through SBUF, take the 2x2 min with two
strided tensor_tensor(min) passes on the vector engine (row pairs, then
column pairs), and stream the 192x192 result back out. Memory-bound:
~94 MB of DMA per core vs ~115us of fully-hidden vector work.
"""

import os

import numpy as np

import concourse.mybir as mybir
from concourse import bacc, bass
from concourse.bass_utils import run_bass_kernel_spmd
from concourse.tile import TileContext

B, C, H, W = 16, 64, 384, 384
NCORES = 8
P = (B // NCORES) * C  # 128 partitions per core
Ho, Wo = H // 2, W // 2
R = 16  # input rows per tile (must be even)
F32 = mybir.dt.float32


def _build() -> bass.Bass:
    nc = bacc.Bacc(None, target_bir_lowering=False, debug=True)
    x = nc.declare_dram_parameter("x", [P, H, W], F32, isOutput=False)
    y = nc.declare_dram_parameter("y", [P, Ho, Wo], F32, isOutput=True)
    GRP = 4  # iterations whose outputs share one output DMA
    with TileContext(nc) as tc:
        with (
            tc.tile_pool(name="tin", bufs=4) as pin,
            tc.tile_pool(name="tmid", bufs=2) as pmid,
            tc.tile_pool(name="tout", bufs=2) as pout,
        ):
            tout = None
            for t in range(H // R):
                tin = pin.tile([P, R, W], F32)
                nc.sync.dma_start(out=tin[:], in_=x[:, t * R : (t + 1) * R, :])
                # min over row pairs: [P, R, W] -> [P, R/2, W]
                v = tin[:].rearrange("p (h two) w -> p h two w", two=2)
                tmid = pmid.tile([P, R // 2, W], F32)
                nc.vector.tensor_tensor(
                    tmid[:], v[:, :, 0, :], v[:, :, 1, :], mybir.AluOpType.min
                )
                # min over column pairs: [P, R/2, W] -> [P, R/2, W/2],
                # accumulated into a GRP-iteration output tile
                if t % GRP == 0:
                    tout = pout.tile([P, GRP * (R // 2), Wo], F32)
                g = t % GRP
                m = tmid[:].rearrange("p h (w two) -> p h w two", two=2)
                nc.vector.tensor_tensor(
                    tout[:, g * (R // 2) : (g + 1) * (R // 2), :],
                    m[:, :, :, 0],
                    m[:, :, :, 1],
                    mybir.AluOpType.min,
                )
                if t % GRP == GRP - 1:
                    base = (t - GRP + 1) * (R // 2)
                    nc.scalar.dma_start(
                        out=y[:, base : base + GRP * (R // 2), :], in_=tout[:]
                    )
    # bass2jax's run_bass_via_pjrt expects a finalized program; for Bacc this
    # also runs compile() (register allocation + sync-wait splitting, which
    # walrus requires: at most one wait per non-event instruction).
    nc.finalize()
    return nc


def kernel(tensor: np.ndarray) -> np.ndarray:
    tensor = np.ascontiguousarray(tensor, dtype=np.float32)
    shards = tensor.reshape(NCORES, P, H, W)  # batch is outermost: 16 -> 8 x 2
    in_maps = [{"x": shards[i]} for i in range(NCORES)]
    nc = _build()
    trace = bool(os.environ.get("GP_TRACE"))
    res = run_bass_kernel_spmd(nc, in_maps, list(range(NCORES)), trace=trace)
    if trace:
        kernel.last_exec_time_ns = res.exec_time_ns
        kernel.last_profile_json = res.profile_json
        kernel.last_trace = res.instructions_and_trace
    out = np.stack([res.results[i]["y"] for i in range(NCORES)])
    return out.reshape(B, C, Ho, Wo)


# revision 7
# speedup vs baseline: 1.2841x; 1.2841x over previous
"""GroupingPool2d kernel for Trainium2 (8 NeuronCores, Bass/Tile).

The reference module (2x2 non-overlapping windows, min-max normalize,
product-group, denormalize) reduces bitwise-exactly to a 2x2 min-pool:
the window minimum normalizes to exactly 0.0, so the product over the
window is exactly 0.0 and out = 0*(mx-mn)+mn = mn.

Strategy: pure data parallel. Shard batch 16 -> 2 per core; per core
flatten (B=2, C=64) -> 128 SBUF partitions, each partition holding one
384x384 image. Stream row-tiles through SBUF, take the 2x2 min with two
strided tensor_tensor(min) passes on the vector engine (row pairs, then
column pairs), and stream the 192x192 result back out. Memory-bound:
~94 MB of DMA per core vs ~115us of fully-hidden vector work.
"""

import os

import numpy as np

import concourse.mybir as mybir
from concourse import bacc, bass
from concourse.bass_utils import run_bass_kernel_spmd
from concourse.tile import TileContext

B, C, H, W = 16, 64, 384, 384
NCORES = 8
P = (B // NCORES) * C  # 128 partitions per core
Ho, Wo = H // 2, W // 2
R = 24  # input rows per tile (must be even)
F32 = mybir.dt.float32


def _build() -> bass.Bass:
    nc = bacc.Bacc(None, target_bir_lowering=False, debug=True)
    x = nc.declare_dram_parameter("x", [P, H, W], F32, isOutput=False)
    y = nc.declare_dram_parameter("y", [P, Ho, Wo], F32, isOutput=True)
    with TileContext(nc) as tc:
        with (
            tc.tile_pool(name="tin", bufs=3) as pin,
            tc.tile_pool(name="tmid", bufs=2) as pmid,
            tc.tile_pool(name="tout", bufs=3) as pout,
        ):
            for t in range(H // R):
                tin = pin.tile([P, R, W], F32)
                nc.sync.dma_start(out=tin[:], in_=x[:, t * R : (t + 1) * R, :])
                # min over column pairs: [P, R, W] -> [P, R, W/2]
                v = tin[:].rearrange("p h (w two) -> p h w two", two=2)
                tmid = pmid.tile([P, R, Wo], F32)
                nc.vector.tensor_tensor(
                    tmid[:], v[:, :, :, 0], v[:, :, :, 1], mybir.AluOpType.min
                )
                # min over row pairs: [P, R, W/2] -> [P, R/2, W/2]
                m = tmid[:].rearrange("p (h two) w -> p h two w", two=2)
                tout = pout.tile([P, R // 2, Wo], F32)
                nc.vector.tensor_tensor(
                    tout[:], m[:, :, 0, :], m[:, :, 1, :], mybir.AluOpType.min
                )
                nc.scalar.dma_start(
                    out=y[:, t * (R // 2) : (t + 1) * (R // 2), :], in_=tout[:]
                )
    # bass2jax's run_bass_via_pjrt expects a finalized program; for Bacc this
    # also runs compile() (register allocation + sync-wait splitting, which
    # walrus requires: at most one wait per non-event instruction).
    nc.finalize()
    return nc


def kernel(tensor: np.ndarray) -> np.ndarray:
    tensor = np.ascontiguousarray(tensor, dtype=np.float32)
    shards = tensor.reshape(NCORES, P, H, W)  # batch is outermost: 16 -> 8 x 2
    in_maps = [{"x": shards[i]} for i in range(NCORES)]
    nc = _build()
    trace = bool(os.environ.get("GP_TRACE"))
    res = run_bass_kernel_spmd(nc, in_maps, list(range(NCORES)), trace=trace)
    if trace:
        kernel.last_exec_time_ns = res.exec_time_ns
        kernel.last_profile_json = res.profile_json
        kernel.last_trace = res.instructions_and_trace
    out = np.stack([res.results[i]["y"] for i in range(NCORES)])
    return out.reshape(B, C, Ho, Wo)
